# revision 56
# baseline (speedup 1.0000x reference)
"""Trainium2 Bass kernel for nn_AttentionLayer (B=4, C=256, N=4096, CR=32).

Sharding: 8 cores = (batch b in 0..3) x (query-half ih in 0..1).
Each core receives x[b] rotated so its own query half sits at columns
0..2047 (softmax is invariant to key order, so the rotation is exact);
it computes out[b][:, ih*2048:(ih+1)*2048] and the host reassembles.

Per-core algorithm (dtype float32r = TF32-class PE inputs, fp32 PSUM):
  - stacked 1x1 conv [Wk; bv@Wk; pad; Wq] @ x -> g (keys), gbv row
    (query-bias correction), h (values)
  - f = Wv @ xq + bv (queries, own half = x columns 0..2047)
  - scores s^T[j,i] = sum_c g_aug[c,j] * f_aug[c,i], K=33 augmented
    contraction ([f;1] x [g;gbv]) so s already includes the query bias.
    Key bias bk is constant over j -> cancels in softmax -> dropped.
    Value bias bq is folded into the output conv bias on the host.
  - exp on ACT, no max subtraction (|s| <~ 40 fits fp32 range)
  - num/den via one accumulating matmul with lhsT = [h^T | 1] per j-tile
  - reciprocal(den), broadcast over partitions via a PE ones-matmul
  - out = (gamma*Wo) @ (num*rden) + (gamma*(Wo@bq + bo)) + x  (bias via
    an exact-ones row in the rhs and a bias row in the weights; the
    residual reads the f32r x tile bitcast back to f32, so it is exact)
"""

import numpy as np

B, C, N = 4, 256, 4096
CR = 32
NH = N // 2          # queries per core
G = 512              # i-group width
NCORES = 8

_CACHE = {}


def build_program():
    """Build the (shared, SPMD) Bass program. Returns compiled nc."""
    import concourse.bacc as bacc
    import concourse.mybir as mybir
    from concourse.tile import TileContext

    dt = mybir.dt
    f32 = dt.float32
    f32r = dt.float32r
    Exp = mybir.ActivationFunctionType.Exp
    add = mybir.AluOpType.add
    mult = mybir.AluOpType.mult

    nc = bacc.Bacc("TRN2", target_bir_lowering=False, debug=False,
                   num_devices=NCORES)

    # --- I/O (all PE operands declared f32r; host passes fp32 bits) ---
    # xw: host-packed [weights(577) | x in piece-major layout (8x1024)].
    # Weight cols: 0-191 wght, 192-255 wft, 256-511 wot(rows 0-63),
    # 512-543 e0, 544-575 idm(rows 0-31), 576 bv. x piece gp, chunk c,
    # col i lives at 577 + gp*1024 + c*512 + i (identical layout in SBUF,
    # so every DMA is a dense contiguous copy and the first one carries
    # the weights and x piece 0 together).
    xw = nc.dram_tensor("xw", [128, 577 + 2 * N], f32r,
                        kind="ExternalInput").ap()
    res = nc.dram_tensor("res", [C, NH], f32, kind="ExternalOutput").ap()

    NJT = N // 128            # 32 j-tiles
    NIG = NH // G             # 4 i-groups
    SUPERS = [3, 3, 3, 3, 3, 3, 3, 3, 3, 3, 2]   # j-tiles per super (sum 32)
    assert sum(SUPERS) == NJT

    with TileContext(nc) as tc:
        with (
            tc.tile_pool(name="const", bufs=1) as cpool,
            tc.tile_pool(name="big", bufs=1) as bpool,
            tc.tile_pool(name="eb", bufs=6) as epool,
            tc.tile_pool(name="small", bufs=3) as spool,
            tc.tile_pool(name="resp", bufs=3) as rpool,
            tc.tile_pool(name="psA", bufs=1, space="PSUM") as psA,
            tc.tile_pool(name="psB", bufs=1, space="PSUM") as psB,
            tc.tile_pool(name="pso", bufs=1, space="PSUM") as pso,
            tc.tile_pool(name="pst", bufs=1, space="PSUM") as pst,
        ):
            # --- weights + x in one identity-layout tile; DMA 0 carries
            # the weights together with x piece 0 ---
            XB = 577
            xall = bpool.tile([128, XB + 2 * N], f32r)
            wght_t = xall[:, 0:192]
            wft_t = xall[:, 192:256]
            wot_t = xall[0:64, 256:512]
            e0_t = xall[:, 512:544]
            idm_t = xall[0:32, 544:576]
            nc.sync.dma_start(xall[:, 0:XB + 1024], xw[:, 0:XB + 1024])
            for gp in range(1, 8):
                s0 = XB + gp * 1024
                nc.sync.dma_start(xall[:, s0:s0 + 1024], xw[:, s0:s0 + 1024])

            def xv(c, col, w):
                # x chunk c, columns [col, col+w) in piece-major layout
                gp = col // G
                assert col % G + w <= G
                return xall[:, XB + gp * 1024 + c * G + col % G:
                            XB + gp * 1024 + c * G + col % G + w]

            # --- activation buffers ---
            f_t = []
            for gi in range(NH // G):
                ft = bpool.tile([128, G], f32r, name=f"f{gi}")
                f_t.append(ft)
                nc.vector.memset(ft[32:33, :].bitcast(f32), 1.0)
            g_aug = bpool.tile([128, N], f32r)    # rows: g(32), gbv(1)
            h_sb = bpool.tile([128, N], f32r)     # rows: h(32)
            hpt = bpool.tile([128, NJT * 33], f32r)  # [h^T | 1] per j-tile

            SPOOLS = (psA, psB)
            SNAMES = ("sa", "sb")

            # --- stacked gh conv: psum rows [g(32); gbv(1); pad; h@64] ---
            GC = 512

            def emit_gh_conv(grp):
                cps = pst.tile([128, GC], f32, name="tl")
                for c in range(2):
                    nc.tensor.matmul(
                        cps[0:96, :],
                        wght_t[:, c * 96:(c + 1) * 96],
                        xv(c, grp * GC, GC),
                        start=(c == 0), stop=(c == 1))
                sl = slice(grp * GC, (grp + 1) * GC)
                nc.vector.tensor_copy(g_aug[0:33, sl], cps[0:33, :])
                nc.vector.tensor_copy(h_sb[0:32, sl], cps[64:96, :])
                emit_gh_tps(grp)

            # transpose a group's 2 h j-tiles into hpt
            def emit_gh_tps(grp):
                tps = pst.tile([128, 128], f32r, name="tlt", tag="tl")
                for k in range(4):
                    t = 4 * grp + k
                    nc.tensor.transpose(
                        tps[:, k * 32:(k + 1) * 32],
                        h_sb[0:32, t * 128:(t + 1) * 128],
                        idm_t)
                hpt_v = hpt[:].rearrange("p (t w) -> p t w", w=33)
                nc.vector.tensor_copy(
                    hpt_v[:, 4 * grp:4 * grp + 4, 0:32],
                    tps[:].rearrange("p (t w) -> p t w", w=32))
                nc.vector.memset(hpt_v[:, 4 * grp:4 * grp + 4, 32:33].bitcast(f32), 1.0)

            # --- f conv (own query half): f = Wv @ xq (bias via gbv row) ---
            def emit_f_conv(fg, pool=None, name="tl"):
                cps = (pool or pst).tile([128, G], f32, name=name, tag=name)
                for c in range(2):
                    nc.tensor.matmul(
                        cps[0:32, :],
                        wft_t[:, c * 32:(c + 1) * 32],
                        xv(c, fg * G, G),
                        start=(c == 0), stop=(c == 1))
                nc.vector.tensor_copy(f_t[fg][0:32, :], cps[0:32, :])

            # --- main attention loop (software-pipelined) ---
            stages = []
            for g in range(NIG):
                jt = 0
                for si, nt in enumerate(SUPERS):
                    stages.append((g, si, jt, nt))
                    jt += nt
            NS = len(stages)

            po_t = {}
            sps_t = {}
            eb_t = {}
            rd_t = {}

            def emit_mm1(idx):
                g, si, jt, nt = stages[idx]
                sps = SPOOLS[idx % 2].tile([128, nt * G], f32,
                                           name=SNAMES[idx % 2])
                sps_t[idx] = sps
                for t in range(nt):
                    nc.tensor.matmul(
                        sps[:, t * G:(t + 1) * G],
                        g_aug[0:33, (jt + t) * 128:(jt + t + 1) * 128],
                        f_t[g][0:33, :],
                        start=True, stop=True)

            def emit_exp(idx):
                g, si, jt, nt = stages[idx]
                eb = epool.tile([128, 3 * G], f32r, name="eb")
                eb_t[idx] = eb
                nc.scalar.activation(
                    eb[:, 0:nt * G], sps_t[idx][:, 0:nt * G], Exp)

            def emit_mm2(idx):
                g, si, jt, nt = stages[idx]
                eb = eb_t.pop(idx)
                sps_t.pop(idx)
                if si == 0:
                    po_t[g] = pso.tile([128, G], f32, name="o")
                for t in range(nt):
                    nc.tensor.matmul(
                        po_t[g][0:33, :],
                        hpt[:, (jt + t) * 33:(jt + t) * 33 + 33],
                        eb[:, t * G:(t + 1) * G],
                        start=(jt + t == 0), stop=(jt + t == NJT - 1))

            def emit_tail_recip(g):
                rd = spool.tile([128, G], f32r, name="rd")
                if g < 3:
                    nc.vector.memset(rd[:].bitcast(f32), 0.0)
                with nc.allow_low_precision(reason="softmax denom"):
                    nc.vector.reciprocal(rd[0:1, :], po_t[g][32:33, :])
                rd_t[g] = rd

            def emit_tail_pe(g, k):
                po = po_t.pop(g)
                rd = rd_t.pop(g)
                bc = pst.tile([128, G], f32, name="tl")
                nc.tensor.matmul(bc[0:32, :], e0_t, rd[:, :],
                                 start=True, stop=True)
                bcs = spool.tile([128, G], f32r, name="bcs")
                nc.vector.tensor_copy(bcs[0:32, :], bc[0:32, :])

                att = spool.tile([128, G], f32r, name="att")
                nc.vector.tensor_tensor(att[0:32, :], po[0:32, :],
                                        bcs[0:32, :], mult)
                if g < 3:
                    nc.vector.memset(att[32:64, :].bitcast(f32), 1.0)

                # output conv (gamma*Wo + bias row) -> + x residual.
                # pf reuses the "o" bank (just freed by att) so the tail
                # never steals a super-pool slot from the mm1 pipeline.
                rt = rpool.tile([128, 1024], f32, name="rt")
                out_v = res.rearrange("(c p) (gg n) -> p gg c n",
                                      c=2, n=G)[:, g]
                for c in range(2):
                    pf = (pst.tile([128, G], f32, name="tl") if c == 0
                          else pso.tile([128, G], f32, name="o"))
                    nc.tensor.matmul(
                        pf[:, :],
                        wot_t[:, c * 128:(c + 1) * 128],
                        att[0:64, :], start=True, stop=True)
                    nc.vector.tensor_tensor(
                        rt[:, c * G:(c + 1) * G], pf[:, :],
                        xv(c, g * G, G).bitcast(f32), add)
                    nc.sync.dma_start(out_v[:, c], rt[:, c * G:(c + 1) * G])

            # Pipeline: mm1[k+1] issues before mm2[k]; gh-conv groups
            # trickle in between igrp-0 stages (DMA-gated anyway); tail PE
            # work is delayed one stage so the reciprocal chain never
            # stalls the PE queue head.
            convs_left = list(range(1, 8))
            f_left = list(range(1, NIG))
            pending_tail = []
            emit_gh_conv(0)
            emit_f_conv(0, pool=pso, name="o")
            emit_mm1(0)
            import os
            KN_FSI = int(os.environ.get("KN_FSI", "6"))
            KN_MM1 = os.environ.get("KN_MM1", "late")
            KN_TDL = int(os.environ.get("KN_TDL", "2"))
            KN_CAH = int(os.environ.get("KN_CAH", "8"))
            for k in range(NS):
                emit_exp(k)
                g, si, jt, nt = stages[k]
                if KN_MM1 == "early" and k + 1 < NS:
                    emit_mm1(k + 1)
                if g == 0:
                    need = min((jt + nt + KN_CAH) // 4, 7)
                    while convs_left and convs_left[0] <= need:
                        emit_gh_conv(convs_left.pop(0))
                if f_left and si >= KN_FSI and f_left[0] <= g + 1:
                    emit_f_conv(f_left.pop(0))
                if KN_MM1 == "mid" and k + 1 < NS:
                    emit_mm1(k + 1)
                if pending_tail and k >= pending_tail[0][1] + KN_TDL:
                    gg, kk = pending_tail.pop(0)
                    emit_tail_pe(gg, k)
                emit_mm2(k)
                if KN_MM1 == "late" and k + 1 < NS:
                    emit_mm1(k + 1)
                if si == len(SUPERS) - 1:
                    emit_tail_recip(g)
                    pending_tail.append((g, k))
            while convs_left:
                emit_gh_conv(convs_left.pop(0))
            while f_left:
                emit_f_conv(f_left.pop(0))
            while pending_tail:
                gg, kk = pending_tail.pop(0)
                emit_tail_pe(gg, kk + 2)

    nc.compile()
    return nc


def _host_prep(Wv, bv, Wk, bk, Wq, bq, Wo, bo, gamma):
    gam = float(np.asarray(gamma).reshape(-1)[0])

    # stacked gh conv weights: rows = [Wk(32); bv@Wk(1); pad(31); Wq(32)]
    w_gh = np.zeros((96, 256), np.float32)
    w_gh[0:32] = Wk
    w_gh[32] = bv @ Wk
    w_gh[64:96] = Wq
    wght = np.zeros((128, 192), np.float32)
    for c in range(2):
        wght[:, c * 96:(c + 1) * 96] = w_gh.T[c * 128:(c + 1) * 128, :]

    wft = np.zeros((128, 64), np.float32)
    for c in range(2):
        wft[:, c * 32:(c + 1) * 32] = Wv.T[c * 128:(c + 1) * 128, :]

    # output conv lhsT rows k: k<32 -> gamma*Wo^T, k==32 -> bias row
    bof = gam * (Wo @ bq + bo)                                  # [256]
    wot = np.zeros((64, 256), np.float32)
    for c in range(2):
        wot[0:32, c * 128:(c + 1) * 128] = gam * Wo[c * 128:(c + 1) * 128, :].T
        wot[32, c * 128:(c + 1) * 128] = bof[c * 128:(c + 1) * 128]

    wpk = np.zeros((128, 577), np.float32)
    wpk[:, 0:192] = wght
    wpk[:, 192:256] = wft
    wpk[0:64, 256:512] = wot
    wpk[0, 512:544] = 1.0                      # e0: ones row
    wpk[0:32, 544:576] = np.eye(32)            # idm
    wpk[0:32, 576] = bv
    return wpk


def kernel(**inputs):
    from concourse.bass_utils import run_bass_kernel_spmd

    x = np.asarray(inputs["x"], np.float32)
    consts = _host_prep(
        np.asarray(inputs["Wv"], np.float32),
        np.asarray(inputs["bv"], np.float32),
        np.asarray(inputs["Wk"], np.float32),
        np.asarray(inputs["bk"], np.float32),
        np.asarray(inputs["Wq"], np.float32),
        np.asarray(inputs["bq"], np.float32),
        np.asarray(inputs["Wo"], np.float32),
        np.asarray(inputs["bo"], np.float32),
        np.asarray(inputs["gamma"], np.float32),
    )

    if "nc" not in _CACHE:
        _CACHE["nc"] = build_program()
    nc = _CACHE["nc"]

    in_maps = []
    for core in range(NCORES):
        b, ih = core // 2, core % 2
        # rotate keys so this core's query half sits at columns 0..NH-1
        # (softmax is invariant to key order, so this is exact), then pack
        # [weights | x] in the kernel's piece-major SBUF layout
        xrot = np.roll(x[b], -ih * NH, axis=1)
        xw = np.empty((128, 577 + 2 * N), np.float32)
        xw[:, 0:577] = consts
        xw[:, 577:] = (xrot.reshape(2, 128, 8, 512)
                       .transpose(1, 2, 0, 3).reshape(128, 2 * N))
        in_maps.append({"xw": xw})

    r = run_bass_kernel_spmd(nc, in_maps, core_ids=list(range(NCORES)),
                             trace=False)
    out = np.empty((B, C, N), np.float32)
    for core in range(NCORES):
        b, ih = core // 2, core % 2
        out[b][:, ih * NH:(ih + 1) * NH] = r.results[core]["res"]
    return out


if __name__ == "__main__":
    nc = build_program()
    print("program built ok")
